# revision 1
# baseline (speedup 1.0000x reference)
"""Multi-head attention TRN2 kernel.

Sharding: 8 cores = 4 batches x 2 head-groups (Megatron tensor parallel over
the 16 heads: Wq/Wk/Wv column-sharded, Wo row-sharded; partial outputs summed
per batch on the host).

Per-core device kernel (batch b, head-group hg -> heads 8hg..8hg+8):
  qT = WqT.T @ xqT            [512, 2048]  (d-major: heads pairwise stacked)
  kT = WkT.T @ xkT            [512, 2048]
  v  = xvT.T  @ WvT           [2048, 512]  + ones column per head
  per (s_tile 512, head-pair, t_chunk 128):
     scoresT[t,s] = kT_h.T @ qT_h     (K=64, heads row-packed 0-63/64-127)
     exp on ACT from PSUM ([128,1024] = both heads), scale=1/sqrt(64)
     ctxT_aug[65,s] += v_aug.T @ expT (ones row accumulates softmax sums)
  normalize: ctxT *= 1/sums (bcast via K=1 matmul), pack into ctxT [512,2048]
  out_partial = ctxT.T @ WoT  [2048, 1024]
"""

import os
import sys
from contextlib import ExitStack

for _p in ("/opt/trn_rl_repo", "/root/.axon_site/_ro/trn_rl_repo"):
    if os.path.isdir(_p) and _p not in sys.path:
        sys.path.insert(0, _p)
        break

import numpy as np

import concourse.bass as bass
import concourse.bacc as bacc
import concourse.mybir as mybir
import concourse.tile as tile

B, S, E, H, D = 4, 2048, 1024, 16, 64
HG = 2          # head groups (tensor-parallel factor)
DH = E // HG    # 512 dims per head group (8 heads)
HPG = H // HG   # 8 heads per group
NCORES = B * HG

F32 = mybir.dt.float32
# matmul operand dtype: float32r streams at ~1 cycle/row (vs 4 for float32)
_MM_DT_NAME = os.environ.get("BASS_MHA_MM_DT", "float32r")
MM_DT = getattr(mybir.dt, _MM_DT_NAME)

SCALE = 1.0 / np.sqrt(D)


def _mm(nc, out, lhsT, rhs, start, stop):
    if lhsT.dtype != MM_DT:
        lhsT = lhsT.bitcast(MM_DT)
    if rhs.dtype != MM_DT:
        rhs = rhs.bitcast(MM_DT)
    nc.tensor.matmul(out, lhsT=lhsT, rhs=rhs, start=start, stop=stop)


def build_nc():
    nc = bacc.Bacc()
    xqT = nc.declare_dram_parameter("xqT", [E, S], MM_DT, isOutput=False)
    xkT = nc.declare_dram_parameter("xkT", [E, S], MM_DT, isOutput=False)
    xvT = nc.declare_dram_parameter("xvT", [E, S], MM_DT, isOutput=False)
    wqT = nc.declare_dram_parameter("wqT", [E, DH], MM_DT, isOutput=False)
    wkT = nc.declare_dram_parameter("wkT", [E, DH], MM_DT, isOutput=False)
    wvT = nc.declare_dram_parameter("wvT", [E, DH], MM_DT, isOutput=False)
    woT = nc.declare_dram_parameter("woT", [DH, E], MM_DT, isOutput=False)
    out = nc.declare_dram_parameter("out", [S, E], F32, isOutput=True)

    with (
        nc.allow_low_precision(reason="float32r matmul operands"),
        tile.TileContext(nc) as tc,
        ExitStack() as ctx,
    ):
        _emit(ctx, tc, xqT, xkT, xvT, wqT, wkT, wvT, woT, out)
    nc.compile()
    return nc


def _final_proj(nc, fps, osb, ctxT_sb, wo_sb, out, s0):
    DC = DH // 128
    for si in range(4):
        r0 = s0 + 128 * si
        o_sb = osb.tile([128, E], F32, tag="osb", name=f"osb_{r0}")
        for et in range(2):
            fp = fps.tile([128, 512], F32, tag="fp", name=f"fp_{r0}_{et}")
            for c in range(DC):
                _mm(
                    nc,
                    fp,
                    ctxT_sb[:, c, r0 : r0 + 128],
                    wo_sb[:, c, 512 * et : 512 * (et + 1)],
                    start=(c == 0),
                    stop=(c == DC - 1),
                )
            nc.vector.tensor_copy(o_sb[:, 512 * et : 512 * (et + 1)], fp)
        nc.sync.dma_start(out=out[r0 : r0 + 128, :], in_=o_sb)


def _emit(ctx, tc, xqT, xkT, xvT, wqT, wkT, wvT, woT, out):
    nc = tc.nc
    EC = E // 128    # 8 contraction chunks for projections
    DC = DH // 128   # 4 d-chunks of the head group
    TC = S // 128    # 16 t chunks
    ST = S // 512    # 4 s tiles
    DA = D + 1       # 65: head dim + ones column

    # ---- persistent tensors ----------------------------------------------
    big = ctx.enter_context(tc.tile_pool(name="big", bufs=1))
    # qT/kT: [p, c, s] with global d = 128*c + p  (head 2c on p 0-63, 2c+1 on 64-127)
    qT_sb = big.tile([128, DC, S], MM_DT, tag="qT")
    kT_sb = big.tile([128, DC, S], MM_DT, tag="kT")
    # v_aug: [t%128, t_chunk, head, 65]; col 64 is the ones column
    v_sb = big.tile([128, TC, HPG, DA], MM_DT, tag="v")
    # ctxT: [p, c, s], hd = 128*c + p
    ctxT_sb = big.tile([128, DC, S], MM_DT, tag="ctx")
    ones_col = big.tile([128, 1], F32, tag="ones_col")
    nc.vector.memset(ones_col, 1.0)
    wo_sb = big.tile([128, DC, E], MM_DT, tag="wo")
    for a in range(DC):
        nc.scalar.dma_start(
            out=wo_sb[:, a, :], in_=woT[128 * a : 128 * (a + 1), :]
        )
    for t in range(TC):
        nc.vector.tensor_copy(
            v_sb[:, t, :, D : D + 1],
            ones_col.to_broadcast((128, HPG)).rearrange("p (h o) -> p h o", o=1),
        )

    # ---- stage B: qT / kT projections ------------------------------------
    # qT[d, s] accumulated over e: lhsT = WqT block [e, d], rhs = xqT [e, s]
    with (
        tc.tile_pool(name="wqk", bufs=1) as wqk,
        tc.tile_pool(name="xin", bufs=5) as xin,
        tc.tile_pool(name="bps", bufs=2, space="PSUM") as bps,
    ):
        wq_sb = wqk.tile([128, EC, DH], MM_DT, tag="wq")
        wk_sb = wqk.tile([128, EC, DH], MM_DT, tag="wk")
        for e in range(EC):
            nc.sync.dma_start(
                out=wq_sb[:, e, :], in_=wqT[128 * e : 128 * (e + 1), :]
            )
            nc.scalar.dma_start(
                out=wk_sb[:, e, :], in_=wkT[128 * e : 128 * (e + 1), :]
            )
        for x_dram, w_sb, dst in ((xqT, wq_sb, qT_sb), (xkT, wk_sb, kT_sb)):
            for sh in range(2):  # s halves of 1024
                ps = []
                for dc in range(DC):
                    ps.append(bps.tile([128, 1024], F32, tag=f"pb{dc}", bufs=1, name=f"pb{dc}_{sh}"))
                for e in range(EC):
                    xtr = xin.tile([128, 1024], MM_DT, tag="xt")
                    eng = nc.sync if x_dram is xqT else nc.scalar
                    eng.dma_start(
                        out=xtr,
                        in_=x_dram[128 * e : 128 * (e + 1), 1024 * sh : 1024 * (sh + 1)],
                    )
                    for dc in range(DC):
                        lhs = w_sb[:, e, 128 * dc : 128 * (dc + 1)]
                        for sq in range(2):
                            _mm(
                                nc,
                                ps[dc][:, 512 * sq : 512 * (sq + 1)],
                                lhs,
                                xtr[:, 512 * sq : 512 * (sq + 1)],
                                start=(e == 0),
                                stop=(e == EC - 1),
                            )
                for dc in range(DC):
                    nc.vector.tensor_copy(
                        dst[:, dc, 1024 * sh : 1024 * (sh + 1)], ps[dc]
                    )

    # ---- stage C: v projection -------------------------------------------
    # v[t, d] accumulated over e: lhsT = xvT block [e, t], rhs = WvT [e, d]
    with (
        tc.tile_pool(name="wvp", bufs=1) as wvp,
        tc.tile_pool(name="xvin", bufs=5) as xvin,
        tc.tile_pool(name="cps", bufs=1, space="PSUM") as cps,
    ):
        wv_sb = wvp.tile([128, EC, DH], MM_DT, tag="wv")
        for e in range(EC):
            nc.gpsimd.dma_start(
                out=wv_sb[:, e, :], in_=wvT[128 * e : 128 * (e + 1), :]
            )
        for th in range(2):  # t halves of 1024
            pv = []
            for tt in range(8):
                pv.append(cps.tile([128, DH], F32, tag=f"pv{tt}", bufs=1, name=f"pv{th}_{tt}"))
            for e in range(EC):
                xtr = xvin.tile([128, 1024], MM_DT, tag="xvt")
                nc.gpsimd.dma_start(
                    out=xtr,
                    in_=xvT[128 * e : 128 * (e + 1), 1024 * th : 1024 * (th + 1)],
                )
                for tt in range(8):
                    _mm(
                        nc,
                        pv[tt],
                        xtr[:, 128 * tt : 128 * (tt + 1)],
                        wv_sb[:, e, :],
                        start=(e == 0),
                        stop=(e == EC - 1),
                    )
            for tt in range(8):
                t = 8 * th + tt
                # strided copy into per-head layout [128, 8, 64]
                nc.vector.tensor_copy(
                    v_sb[:, t, :, 0:D],
                    pv[tt].rearrange("p (h d) -> p h d", h=HPG),
                )

    # ---- stage D/E: attention + output projection ------------------------
    with (
        tc.tile_pool(name="ex", bufs=4) as expool,
        tc.tile_pool(name="small", bufs=3) as small,
        tc.tile_pool(name="osb", bufs=3) as osb,
        tc.tile_pool(name="dps", bufs=2, space="PSUM") as dps,
        tc.tile_pool(name="cxps", bufs=3, space="PSUM") as cxps,
        tc.tile_pool(name="fps", bufs=1, space="PSUM") as fps,
    ):
        for st in range(ST):
            s0 = 512 * st
            for c in range(DC):  # head pair (2c, 2c+1)
                cx = [cxps.tile([DA, 512], F32, tag="cx", name=f"cx{st}_{c}_{j2}") for j2 in range(2)]
                for t in range(TC):
                    sc = dps.tile([128, 1024], F32, tag="sc")
                    for j in range(2):
                        _mm(
                            nc,
                            sc[:, 512 * j : 512 * (j + 1)],
                            kT_sb[64 * j : 64 * (j + 1), c, 128 * t : 128 * (t + 1)],
                            qT_sb[64 * j : 64 * (j + 1), c, s0 : s0 + 512],
                            start=True,
                            stop=True,
                        )
                    ex = expool.tile([128, 1024], MM_DT, tag="ex")
                    nc.scalar.activation(
                        out=ex,
                        in_=sc,
                        func=mybir.ActivationFunctionType.Exp,
                        scale=float(SCALE),
                    )
                    for j in range(2):
                        _mm(
                            nc,
                            cx[j],
                            v_sb[:, t, 2 * c + j, :],
                            ex[:, 512 * j : 512 * (j + 1)],
                            start=(t == 0),
                            stop=(t == TC - 1),
                        )
                # free cx psum slots fast, normalize from SBUF off-path
                for j in range(2):
                    cxs = small.tile([DA, 512], F32, tag="cxs", name=f"cxs{st}_{c}_{j}")
                    nc.vector.tensor_copy(cxs, cx[j])
                    rec = small.tile([1, 512], F32, tag="rec")
                    nc.vector.reciprocal(rec, cxs[D : D + 1, :])
                    bc_sb = small.tile([64, 512], F32, tag="bcsb")
                    nc.gpsimd.partition_broadcast(bc_sb, rec)
                    nc.vector.tensor_mul(
                        ctxT_sb[64 * j : 64 * (j + 1), c, s0 : s0 + 512],
                        cxs[0:D, :],
                        bc_sb,
                    )
            # output projection for the PREVIOUS s-tile (hides the norm chain)
            if st > 0:
                _final_proj(nc, fps, osb, ctxT_sb, wo_sb, out, 512 * (st - 1))
        _final_proj(nc, fps, osb, ctxT_sb, wo_sb, out, 512 * (ST - 1))


_BUILT = {}


def _get_nc():
    if "nc" not in _BUILT:
        _BUILT["nc"] = build_nc()
    return _BUILT["nc"]


def make_in_maps(query, key, value, Wq, Wk, Wv, Wo):
    ndt = mybir.dt.np(MM_DT)
    query = np.asarray(query, np.float32).astype(ndt)
    key = np.asarray(key, np.float32).astype(ndt)
    value = np.asarray(value, np.float32).astype(ndt)
    Wq = np.asarray(Wq, np.float32).astype(ndt)
    Wk = np.asarray(Wk, np.float32).astype(ndt)
    Wv = np.asarray(Wv, np.float32).astype(ndt)
    Wo = np.asarray(Wo, np.float32).astype(ndt)

    xqT = [np.ascontiguousarray(query[b].T) for b in range(B)]
    xkT = [np.ascontiguousarray(key[b].T) for b in range(B)]
    xvT = [np.ascontiguousarray(value[b].T) for b in range(B)]
    wqT = [np.ascontiguousarray(Wq[DH * g : DH * (g + 1), :].T) for g in range(HG)]
    wkT = [np.ascontiguousarray(Wk[DH * g : DH * (g + 1), :].T) for g in range(HG)]
    wvT = [np.ascontiguousarray(Wv[DH * g : DH * (g + 1), :].T) for g in range(HG)]
    woT = [np.ascontiguousarray(Wo[:, DH * g : DH * (g + 1)].T) for g in range(HG)]

    in_maps = []
    for core in range(NCORES):
        b, g = core // HG, core % HG
        in_maps.append(
            {
                "xqT": xqT[b],
                "xkT": xkT[b],
                "xvT": xvT[b],
                "wqT": wqT[g],
                "wkT": wkT[g],
                "wvT": wvT[g],
                "woT": woT[g],
            }
        )
    return in_maps


def assemble(core_outs):
    out = np.empty((B, S, E), np.float32)
    for b in range(B):
        out[b] = core_outs[HG * b]
        for g in range(1, HG):
            out[b] += core_outs[HG * b + g]
    return out


def kernel(query, key, value, Wq, Wk, Wv, Wo):
    from concourse.bass_utils import run_bass_kernel_spmd

    nc = _get_nc()
    in_maps = make_in_maps(query, key, value, Wq, Wk, Wv, Wo)
    res = run_bass_kernel_spmd(nc, in_maps, list(range(NCORES)))
    return assemble([r["out"] for r in res.results])



# revision 5
# speedup vs baseline: 1.4471x; 1.4471x over previous
"""Multi-head attention TRN2 kernel.

Sharding: 8 cores = 4 batches x 2 head-groups (Megatron tensor parallel over
the 16 heads: Wq/Wk/Wv column-sharded, Wo row-sharded; partial outputs summed
per batch on the host).

Per-core device kernel (batch b, head-group hg -> heads 8hg..8hg+8):
  v  = xvT.T  @ WvT           [2048, 512]  + ones column per head
  qT = WqT.T @ xqT            [512, 2048]  (d-major: heads pairwise stacked)
  kT = WkT.T @ xkT            [512, 2048]
  per (s_tile 512, head-pair, t_chunk 128):
     scoresT[t,s] = kT_h.T @ qT_h     (K=64, heads row-packed 0-63/64-127)
     exp on ACT from PSUM ([128,1024] = both heads), scale=1/sqrt(64)
     ctxT_aug[65,s] += v_aug.T @ expT (ones row accumulates softmax sums)
  normalize: PSUM-freeing copies first, then approx-reciprocal (DVE custom op)
  + gpsimd broadcast/mul off the critical path -> ctxT [512,2048]
  out_partial = ctxT.T @ WoT  [2048, 1024]
"""

import os
import sys
from contextlib import ExitStack

for _p in ("/opt/trn_rl_repo", "/root/.axon_site/_ro/trn_rl_repo"):
    if os.path.isdir(_p) and _p not in sys.path:
        sys.path.insert(0, _p)
        break

import numpy as np

import concourse.bass as bass
import concourse.bacc as bacc
import concourse.mybir as mybir
import concourse.tile as tile

B, S, E, H, D = 4, 2048, 1024, 16, 64
HG = 2          # head groups (tensor-parallel factor)
DH = E // HG    # 512 dims per head group (8 heads)
HPG = H // HG   # 8 heads per group
NCORES = B * HG

F32 = mybir.dt.float32
# matmul operand dtype: bf16 streams at 1 col/cycle, halves SBUF+DMA traffic
_MM_DT_NAME = os.environ.get("BASS_MHA_MM_DT", "bfloat16")
MM_DT = getattr(mybir.dt, _MM_DT_NAME)

SCALE = 1.0 / np.sqrt(D)


def _mm(nc, out, lhsT, rhs, start, stop):
    if lhsT.dtype != MM_DT:
        lhsT = lhsT.bitcast(MM_DT)
    if rhs.dtype != MM_DT:
        rhs = rhs.bitcast(MM_DT)
    nc.tensor.matmul(out, lhsT=lhsT, rhs=rhs, start=start, stop=stop)


def build_nc():
    nc = bacc.Bacc()
    xqT = nc.declare_dram_parameter("xqT", [E, S], MM_DT, isOutput=False)
    xkT = nc.declare_dram_parameter("xkT", [E, S], MM_DT, isOutput=False)
    xvT = nc.declare_dram_parameter("xvT", [E, S], MM_DT, isOutput=False)
    wqT = nc.declare_dram_parameter("wqT", [E, DH], MM_DT, isOutput=False)
    wkT = nc.declare_dram_parameter("wkT", [E, DH], MM_DT, isOutput=False)
    wvT = nc.declare_dram_parameter("wvT", [E, DH], MM_DT, isOutput=False)
    woT = nc.declare_dram_parameter("woT", [DH, E], MM_DT, isOutput=False)
    out = nc.declare_dram_parameter("out", [S, E], F32, isOutput=True)

    with (
        nc.allow_low_precision(reason="bf16 matmul operands"),
        tile.TileContext(nc) as tc,
        ExitStack() as ctx,
    ):
        _emit(ctx, tc, xqT, xkT, xvT, wqT, wkT, wvT, woT, out)
    nc.compile()
    return nc


def _final_proj(nc, fps, osb, ctxT_sb, wo_sb, out, s0):
    DC = DH // 128
    for si in range(4):
        r0 = s0 + 128 * si
        o_sb = osb.tile([128, E], F32, tag="osb", name=f"osb_{r0}")
        for et in range(2):
            fp = fps.tile([128, 512], F32, tag="fp", name=f"fp_{r0}_{et}")
            for c in range(DC):
                _mm(
                    nc,
                    fp,
                    ctxT_sb[:, c, r0 : r0 + 128],
                    wo_sb[:, c, 512 * et : 512 * (et + 1)],
                    start=(c == 0),
                    stop=(c == DC - 1),
                )
            nc.vector.tensor_copy(o_sb[:, 512 * et : 512 * (et + 1)], fp)
        nc.sync.dma_start(out=out[r0 : r0 + 128, :], in_=o_sb)


def _emit(ctx, tc, xqT, xkT, xvT, wqT, wkT, wvT, woT, out):
    nc = tc.nc
    EC = E // 128    # 8 contraction chunks for projections
    DC = DH // 128   # 4 d-chunks of the head group
    TC = S // 128    # 16 t chunks
    ST = S // 512    # 4 s tiles
    DA = D + 1       # 65: head dim + ones column

    # ---- persistent tensors ----------------------------------------------
    big = ctx.enter_context(tc.tile_pool(name="big", bufs=1))
    # qT/kT: [p, c, s] with global d = 128*c + p  (head 2c on p 0-63, 2c+1 on 64-127)
    qT_sb = big.tile([128, DC, S], MM_DT, tag="qT")
    kT_sb = big.tile([128, DC, S], MM_DT, tag="kT")
    # v_aug: [t%128, t_chunk, head, 65]; col 64 is the ones column
    v_sb = big.tile([128, TC, HPG, DA], MM_DT, tag="v")
    # ctxT: [p, c, s], hd = 128*c + p
    ctxT_sb = big.tile([128, DC, S], MM_DT, tag="ctx")
    ones_col = big.tile([128, 1], F32, tag="ones_col")
    nc.vector.memset(ones_col, 1.0)
    wo_sb = big.tile([128, DC, E], MM_DT, tag="wo")
    for a in range(DC):
        nc.scalar.dma_start(
            out=wo_sb[:, a, :], in_=woT[128 * a : 128 * (a + 1), :]
        )
    for t in range(TC):
        nc.vector.tensor_copy(
            v_sb[:, t, :, D : D + 1],
            ones_col.to_broadcast((128, HPG)).rearrange("p (h o) -> p h o", o=1),
        )

    # ---- stage C: v projection (first: its DMA overlaps the q/k stages) ---
    # v[t, d] accumulated over e: lhsT = xvT block [e, t], rhs = WvT [e, d]
    with (
        tc.tile_pool(name="wvp", bufs=1) as wvp,
        tc.tile_pool(name="xvin", bufs=5) as xvin,
        tc.tile_pool(name="cps", bufs=1, space="PSUM") as cps,
    ):
        wv_sb = wvp.tile([128, EC, DH], MM_DT, tag="wv")
        for e in range(EC):
            nc.gpsimd.dma_start(
                out=wv_sb[:, e, :], in_=wvT[128 * e : 128 * (e + 1), :]
            )
        for th in range(2):  # t halves of 1024
            pv = []
            for tt in range(8):
                pv.append(cps.tile([128, DH], F32, tag=f"pv{tt}", bufs=1, name=f"pv{th}_{tt}"))
            for e in range(EC):
                xtr = xvin.tile([128, 1024], MM_DT, tag="xvt")
                nc.gpsimd.dma_start(
                    out=xtr,
                    in_=xvT[128 * e : 128 * (e + 1), 1024 * th : 1024 * (th + 1)],
                )
                for tt in range(8):
                    _mm(
                        nc,
                        pv[tt],
                        xtr[:, 128 * tt : 128 * (tt + 1)],
                        wv_sb[:, e, :],
                        start=(e == 0),
                        stop=(e == EC - 1),
                    )
            for tt in range(8):
                t = 8 * th + tt
                # strided copy into per-head layout [128, 8, 64]
                nc.vector.tensor_copy(
                    v_sb[:, t, :, 0:D],
                    pv[tt].rearrange("p (h d) -> p h d", h=HPG),
                )

    # ---- stage B: qT / kT projections ------------------------------------
    # qT[d, s] accumulated over e: lhsT = WqT block [e, d], rhs = xqT [e, s]
    with (
        tc.tile_pool(name="wqk", bufs=1) as wqk,
        tc.tile_pool(name="xin", bufs=5) as xin,
        tc.tile_pool(name="bps", bufs=2, space="PSUM") as bps,
    ):
        wq_sb = wqk.tile([128, EC, DH], MM_DT, tag="wq")
        wk_sb = wqk.tile([128, EC, DH], MM_DT, tag="wk")
        for e in range(EC):
            nc.sync.dma_start(
                out=wq_sb[:, e, :], in_=wqT[128 * e : 128 * (e + 1), :]
            )
            nc.scalar.dma_start(
                out=wk_sb[:, e, :], in_=wkT[128 * e : 128 * (e + 1), :]
            )
        for x_dram, w_sb, dst in ((xkT, wk_sb, kT_sb), (xqT, wq_sb, qT_sb)):
            for sh in range(2):  # s halves of 1024
                ps = []
                for dc in range(DC):
                    ps.append(bps.tile([128, 1024], F32, tag=f"pb{dc}", bufs=1, name=f"pb{dc}_{sh}"))
                for e in range(EC):
                    xtr = xin.tile([128, 1024], MM_DT, tag="xt")
                    eng = nc.sync if x_dram is xqT else nc.scalar
                    eng.dma_start(
                        out=xtr,
                        in_=x_dram[128 * e : 128 * (e + 1), 1024 * sh : 1024 * (sh + 1)],
                    )
                    for dc in range(DC):
                        lhs = w_sb[:, e, 128 * dc : 128 * (dc + 1)]
                        for sq in range(2):
                            _mm(
                                nc,
                                ps[dc][:, 512 * sq : 512 * (sq + 1)],
                                lhs,
                                xtr[:, 512 * sq : 512 * (sq + 1)],
                                start=(e == 0),
                                stop=(e == EC - 1),
                            )
                for dc in range(DC):
                    nc.vector.tensor_copy(
                        dst[:, dc, 1024 * sh : 1024 * (sh + 1)], ps[dc]
                    )

    # ---- stage D/E: attention + output projection ------------------------
    with (
        tc.tile_pool(name="ex", bufs=4) as expool,
        tc.tile_pool(name="small", bufs=3) as small,
        tc.tile_pool(name="osb", bufs=3) as osb,
        tc.tile_pool(name="dps", bufs=2, space="PSUM") as dps,
        tc.tile_pool(name="cxps", bufs=3, space="PSUM") as cxps,
        tc.tile_pool(name="fps", bufs=1, space="PSUM") as fps,
    ):
        for st in range(ST):
            s0 = 512 * st
            for c in range(DC):  # head pair (2c, 2c+1)
                cx = [cxps.tile([DA, 512], F32, tag="cx", name=f"cx{st}_{c}_{j2}") for j2 in range(2)]
                for t in range(TC):
                    sc = dps.tile([128, 1024], F32, tag="sc")
                    for j in range(2):
                        _mm(
                            nc,
                            sc[:, 512 * j : 512 * (j + 1)],
                            kT_sb[64 * j : 64 * (j + 1), c, 128 * t : 128 * (t + 1)],
                            qT_sb[64 * j : 64 * (j + 1), c, s0 : s0 + 512],
                            start=True,
                            stop=True,
                        )
                    ex = expool.tile([128, 1024], MM_DT, tag="ex")
                    nc.scalar.activation(
                        out=ex,
                        in_=sc,
                        func=mybir.ActivationFunctionType.Exp,
                        scale=float(SCALE),
                    )
                    for j in range(2):
                        _mm(
                            nc,
                            cx[j],
                            v_sb[:, t, 2 * c + j, :],
                            ex[:, 512 * j : 512 * (j + 1)],
                            start=(t == 0),
                            stop=(t == TC - 1),
                        )
                # free cx psum slots fast: both PSUM->SBUF copies come first,
                # then the (cheap) approx reciprocal; broadcast+mul on gpsimd.
                cxs = []
                for j in range(2):
                    cxs.append(small.tile([DA, 512], F32, tag="cxs", bufs=4, name=f"cxs{st}_{c}_{j}"))
                    nc.vector.tensor_copy(cxs[j], cx[j])
                for j in range(2):
                    # reciprocal_approx_fast mishandles base_partition != 0:
                    # stage the sums row down to partition 0 first.
                    sums_st = small.tile([1, 512], F32, tag="sums_st")
                    nc.vector.tensor_copy(sums_st, cxs[j][D : D + 1, :])
                    rec = small.tile([1, 512], F32, tag="rec")
                    nc.vector.reciprocal_approx_fast(out=rec, in_=sums_st)
                    bc_sb = small.tile([64, 512], F32, tag="bcsb")
                    nc.gpsimd.partition_broadcast(bc_sb, rec)
                    nc.vector.tensor_mul(
                        ctxT_sb[64 * j : 64 * (j + 1), c, s0 : s0 + 512],
                        cxs[j][0:D, :],
                        bc_sb,
                    )
            # output projection for the PREVIOUS s-tile (hides the norm chain)
            if st > 0:
                _final_proj(nc, fps, osb, ctxT_sb, wo_sb, out, 512 * (st - 1))
        _final_proj(nc, fps, osb, ctxT_sb, wo_sb, out, 512 * (ST - 1))


_BUILT = {}


def _get_nc():
    if "nc" not in _BUILT:
        _BUILT["nc"] = build_nc()
    return _BUILT["nc"]


def make_in_maps(query, key, value, Wq, Wk, Wv, Wo):
    ndt = mybir.dt.np(MM_DT)
    query = np.asarray(query, np.float32).astype(ndt)
    key = np.asarray(key, np.float32).astype(ndt)
    value = np.asarray(value, np.float32).astype(ndt)
    Wq = np.asarray(Wq, np.float32).astype(ndt)
    Wk = np.asarray(Wk, np.float32).astype(ndt)
    Wv = np.asarray(Wv, np.float32).astype(ndt)
    Wo = np.asarray(Wo, np.float32).astype(ndt)

    xqT = [np.ascontiguousarray(query[b].T) for b in range(B)]
    xkT = [np.ascontiguousarray(key[b].T) for b in range(B)]
    xvT = [np.ascontiguousarray(value[b].T) for b in range(B)]
    wqT = [np.ascontiguousarray(Wq[DH * g : DH * (g + 1), :].T) for g in range(HG)]
    wkT = [np.ascontiguousarray(Wk[DH * g : DH * (g + 1), :].T) for g in range(HG)]
    wvT = [np.ascontiguousarray(Wv[DH * g : DH * (g + 1), :].T) for g in range(HG)]
    woT = [np.ascontiguousarray(Wo[:, DH * g : DH * (g + 1)].T) for g in range(HG)]

    in_maps = []
    for core in range(NCORES):
        b, g = core // HG, core % HG
        in_maps.append(
            {
                "xqT": xqT[b],
                "xkT": xkT[b],
                "xvT": xvT[b],
                "wqT": wqT[g],
                "wkT": wkT[g],
                "wvT": wvT[g],
                "woT": woT[g],
            }
        )
    return in_maps


def assemble(core_outs):
    out = np.empty((B, S, E), np.float32)
    for b in range(B):
        out[b] = core_outs[HG * b]
        for g in range(1, HG):
            out[b] += core_outs[HG * b + g]
    return out


def kernel(query, key, value, Wq, Wk, Wv, Wo):
    from concourse.bass_utils import run_bass_kernel_spmd

    nc = _get_nc()
    in_maps = make_in_maps(query, key, value, Wq, Wk, Wv, Wo)
    res = run_bass_kernel_spmd(nc, in_maps, list(range(NCORES)))
    return assemble([r["out"] for r in res.results])
